# revision 19
# baseline (speedup 1.0000x reference)
"""Trainium2 Bass kernel for nn_Attention_39608188404100.

Windowed-attention block (ViT-style, N=197 tokens) with SSF affines, relative
position bias, DCF head mixing, and output projection.

Strategy: pure data-parallel over batch across 8 NeuronCores (B=64 -> 8/core).
All weights replicated; no collectives. Compute in bf16 on the TensorEngine
(fp32 PSUM accumulation). fp8 was tried and rejected: each e4m3 quantization
point adds ~3.6% RMS error, far over the 2% tolerance.

Per core (BL=8 batches): each batch's 197 tokens are padded to 200 positions
and PERMUTED on host: position p = c*100 + ml*10 + g holds token
m = c*100 + g*10 + ml (c = chunk, 2x100). The 3 dummy positions per batch get
zero x-columns and a -40 relative-bias on their key rows; dummy query columns
are dropped on host after download.

Pipeline: fully interleaved at batch-PAIR granularity. For each pair-block:
load x columns -> QKV projection (N=400 matmuls) -> V -> per batch: scores,
exp, softmax epilogue, keypos<->(wgi,h) shuffle (10 direct SBUF->SBUF DMAs
each way), block-diagonal [120x120] DCF mix, AV, output projection. This
keeps the TensorEngine fed throughout instead of a dense QKV phase followed
by a sparse attention phase, and lets x/qk/v tiles be pair-sized (double
buffered) instead of whole-core.

Softmax denominator: ones-column matmul into PSUM, in-place fp32
reciprocal_approx_fast on the PSUM row, bf16 row, DRAM-bounce broadcast DMA
to 100 partitions, two in-place normalize multiplies.
"""
import os
import sys

sys.path.insert(0, "/opt/trn_rl_repo")

import numpy as np
import ml_dtypes

import concourse.bass as bass
import concourse.tile as tile
from concourse import bacc, mybir

BF16 = mybir.dt.bfloat16
F32 = mybir.dt.float32
AF = mybir.ActivationFunctionType
ALU = mybir.AluOpType

B, N, C, H = 64, 197, 768, 12
NCORES = 8
BL = B // NCORES          # 8 batches per core
P2 = 200                  # padded positions per batch
T2 = BL * P2              # 1600 positions per core
PW = 2 * P2               # pair width (400 token columns)
SCALE = (C // H) ** -0.5
KT = 6                    # contraction tiles of 128 over C=768
QKM = 12                  # 128-wide M tiles over 1536 q/k channels
DUMMY_BIAS = -40.0
DEN_CH = 480              # denominator column chunk (5 x 480 = H*P2)

_COMPILED = {}


def _build_graph():
    nc = bacc.Bacc(
        "TRN2", target_bir_lowering=False, debug=False,
        detect_race_conditions=False,
    )

    xT_d = nc.dram_tensor("xT", [128, KT * T2], BF16, kind="ExternalInput")
    wqk_d = nc.dram_tensor("wqk", [128, KT * 1536], BF16, kind="ExternalInput")
    wv_d = nc.dram_tensor("wv", [128, KT * 768], BF16, kind="ExternalInput")
    wp_d = nc.dram_tensor("wp", [128, KT * 768], BF16, kind="ExternalInput")
    relb_d = nc.dram_tensor("relb", [100, 2 * H * P2], BF16, kind="ExternalInput")
    mix_d = nc.dram_tensor("mixblk", [120, 120], BF16, kind="ExternalInput")
    bqk_d = nc.dram_tensor("bqk", [128, QKM], F32, kind="ExternalInput")
    bv_d = nc.dram_tensor("bv", [1, 768], BF16, kind="ExternalInput")
    bp_d = nc.dram_tensor("bp", [1, 768], BF16, kind="ExternalInput")
    out_d = nc.dram_tensor("out", [T2, 768], BF16, kind="ExternalOutput")

    with tile.TileContext(nc) as tc:
        with (
            tc.tile_pool(name="const", bufs=1) as cpool,
            tc.tile_pool(name="xt", bufs=2) as xtpool,
            tc.tile_pool(name="qk", bufs=2) as qkpool,
            tc.tile_pool(name="vv", bufs=2) as vpool,
            tc.tile_pool(name="exp", bufs=3) as exppool,
            tc.tile_pool(name="small", bufs=2) as smallpool,
            tc.tile_pool(name="a2", bufs=2) as a2pool,
            tc.tile_pool(name="mxin", bufs=2) as mxpool,
            tc.tile_pool(name="mxout", bufs=2) as mopool,
            tc.tile_pool(name="ao", bufs=2) as aopool,
            tc.tile_pool(name="denb2", bufs=2) as dbpool,
            tc.tile_pool(name="osb", bufs=2) as opool,
            tc.tile_pool(name="dram", bufs=2, space=bass.MemorySpace.DRAM) as drpool,
            tc.tile_pool(name="psA", bufs=2, space=bass.MemorySpace.PSUM) as psA,
            tc.tile_pool(name="psS", bufs=2, space=bass.MemorySpace.PSUM) as psS,
            tc.tile_pool(name="psD", bufs=2, space=bass.MemorySpace.PSUM) as psD,
            tc.tile_pool(name="psMV", bufs=2, space=bass.MemorySpace.PSUM) as psMV,
        ):
            # ---- persistent constants / weights ----
            wqk = cpool.tile([128, KT * 1536], BF16)
            wv = cpool.tile([128, KT * 768], BF16)
            wp = cpool.tile([128, KT * 768], BF16)
            relb = cpool.tile([100, 2 * H * P2], BF16)
            mixblk = cpool.tile([120, 120], BF16)
            bqk = cpool.tile([128, QKM], F32)
            bv = cpool.tile([1, 768], BF16)
            bp = cpool.tile([1, 768], BF16)
            ones_col = cpool.tile([128, 1], BF16)   # lhsT for denominator
            ones_row = cpool.tile([1, 128], BF16)   # lhsT for rank-1 bias
            for kt in range(KT):
                nc.sync.dma_start(
                    wqk[:, kt * 1536 : (kt + 1) * 1536],
                    wqk_d[:, kt * 1536 : (kt + 1) * 1536],
                )
            nc.sync.dma_start(wv[:], wv_d[:])
            nc.sync.dma_start(wp[:], wp_d[:])
            nc.sync.dma_start(relb[:], relb_d[:])
            nc.sync.dma_start(mixblk[:], mix_d[:])
            nc.sync.dma_start(bqk[:], bqk_d[:])
            nc.sync.dma_start(bv[:], bv_d[:])
            nc.sync.dma_start(bp[:], bp_d[:])
            nc.vector.memset(ones_col[:], 1.0)
            nc.vector.memset(ones_row[:], 1.0)

            def emit_front(b, b2, expAll, qkt):
                """Scores, exp, softmax epilogue, in-place normalize."""
                eP = expAll[:].rearrange("p (h two n) -> p h two n",
                                         h=H, two=2, n=P2)
                for h in range(H):
                    prow = (h % 2) * 64
                    qoff = (h // 2) * PW + b2 * P2
                    koff = (6 + h // 2) * PW + b2 * P2

                    ps1 = psS.tile([128, 512], F32, tag="s")
                    nc.tensor.matmul(
                        ps1[0:100, 0:P2],
                        qkt[prow : prow + 64, koff : koff + 100],
                        qkt[prow : prow + 64, qoff : qoff + P2],
                        start=True, stop=True,
                    )
                    nc.tensor.matmul(
                        ps1[0:100, P2 : 2 * P2],
                        qkt[prow : prow + 64, koff + 100 : koff + 200],
                        qkt[prow : prow + 64, qoff : qoff + P2],
                        start=True, stop=True,
                    )
                    nc.scalar.activation(eP[0:100, h], ps1[0:100, 0 : 2 * P2],
                                         AF.Exp)

                # softmax epilogue: exp(rel-bias) multiply + chunk-sum in two
                # head-halves, ones-matmul denominator per 480-col chunk,
                # in-place fp32 fast reciprocal in PSUM, bf16 row, DRAM-bounce
                # broadcast, two in-place normalize multiplies.
                ev = eP[0:100]                              # [100, h, two, n]
                rv = relb[0:100, :].rearrange("p (h c n) -> p h c n", h=H, c=2, n=P2)
                denb = smallpool.tile([100, H * P2], BF16, tag="denb")
                dv = denb[:].rearrange("p (h n) -> p h n", h=H)
                for h0 in (0, 6):
                    nc.vector.tensor_tensor(ev[:, h0 : h0 + 6], ev[:, h0 : h0 + 6],
                                            rv[:, h0 : h0 + 6], ALU.mult)
                    nc.vector.tensor_tensor(dv[:, h0 : h0 + 6],
                                            ev[:, h0 : h0 + 6, 0, :],
                                            ev[:, h0 : h0 + 6, 1, :], ALU.add)
                denr = dbpool.tile([1, H * P2], BF16, tag="denr")
                for o in range(0, H * P2, DEN_CH):
                    psd = psD.tile([128, 512], F32, tag="d")
                    nc.tensor.matmul(psd[0:1, 0:DEN_CH], ones_col[0:100, 0:1],
                                     denb[0:100, o : o + DEN_CH],
                                     start=True, stop=True)
                    nc.vector.reciprocal_approx_fast(psd[0:1, 0:DEN_CH],
                                                     psd[0:1, 0:DEN_CH])
                    with nc.allow_low_precision(reason="softmax recip in bf16"):
                        nc.vector.tensor_copy(denr[0:1, o : o + DEN_CH],
                                              psd[0:1, 0:DEN_CH])
                ddr = drpool.tile([1, H * P2], BF16, tag="ddr")
                nc.gpsimd.dma_start(ddr[:], denr[:])
                denbc = dbpool.tile([100, H * P2], BF16, tag="denbc")
                nc.gpsimd.dma_start(denbc[:], ddr[:].to_broadcast([100, H * P2]))
                dvb = denbc[:].rearrange("p (h n) -> p h n", h=H)
                nc.vector.tensor_tensor(ev[:, :, 0, :], ev[:, :, 0, :], dvb, ALU.mult)
                nc.vector.tensor_tensor(ev[:, :, 1, :], ev[:, :, 1, :], dvb, ALU.mult)

            def emit_mix(expAll):
                """keypos->(wgi,h) shuffle, block-diag mix, shuffle back.
                mxin[wgi*12+h, j*400+(two n)] = expAll[j*10+wgi, h*400+(two n)]"""
                mxin = mxpool.tile([120, 10 * PW], BF16, tag="mxin")
                for j in range(10):
                    eng = nc.sync if j % 2 == 0 else nc.gpsimd
                    eng.dma_start(mxin[:, j * 400 : (j + 1) * 400],
                                  expAll[j * 10 : (j + 1) * 10, :])
                mxo = mopool.tile([120, 10 * PW], BF16, tag="mxout")
                for o in range(0, 4000, 500):
                    psm = psMV.tile([128, 512], F32, tag="mv")
                    nc.tensor.matmul(
                        psm[0:120, 0:500], mixblk[:],
                        mxin[:, o : o + 500],
                        start=True, stop=True,
                    )
                    nc.scalar.copy(mxo[:, o : o + 500], psm[0:120, 0:500])
                a2 = a2pool.tile([100, 2 * H * P2], BF16)  # [keypos, (k,two,n)]
                for j in range(10):
                    eng = nc.sync if j % 2 == 0 else nc.gpsimd
                    eng.dma_start(a2[j * 10 : (j + 1) * 10, :],
                                  mxo[:, j * 400 : (j + 1) * 400])
                return a2

            def emit_back(b, b2, a2, vt):
                """AV + projection for one batch."""
                a2v = a2[:].rearrange("p (k two n) -> p k two n",
                                      k=H, two=2, n=P2)
                aoT = aopool.tile([128, KT * P2], BF16, tag="ao")
                for tt in range(H // 4):        # 3 psum tiles, 2 head-pairs each
                    pv = psMV.tile([128, 512], F32, tag="mv")
                    for jj2 in range(2):
                        jj = tt * 2 + jj2
                        for sub in range(2):
                            k = 2 * jj + sub
                            rows = pv[sub * 64 : sub * 64 + 64,
                                      jj2 * P2 : (jj2 + 1) * P2]
                            for c in range(2):
                                nc.tensor.matmul(
                                    rows,
                                    vt[0:100, (b2 * 2 + c) * 768 + k * 64 : (b2 * 2 + c) * 768 + (k + 1) * 64],
                                    a2v[0:100, k, c],
                                    start=(c == 0),
                                    stop=(c == 1),
                                    tile_position=(0, sub * 64),
                                )
                    nc.scalar.copy(aoT[:, tt * 2 * P2 : (tt + 1) * 2 * P2],
                                   pv[:, 0 : 2 * P2])

                for (t0, tsz) in [(0, 128), (128, 72)]:
                    osb = opool.tile([128, 768], BF16, tag="osb")
                    for (n0, nsz) in [(0, 512), (512, 256)]:
                        pp = psA.tile([128, 512], F32, tag="a")
                        nc.tensor.matmul(
                            pp[0:tsz, 0:nsz],
                            ones_row[0:1, 0:tsz],
                            bp[:, n0 : n0 + nsz],
                            start=True, stop=False,
                        )
                        for kt in range(KT):
                            nc.tensor.matmul(
                                pp[0:tsz, 0:nsz],
                                aoT[:, kt * P2 + t0 : kt * P2 + t0 + tsz],
                                wp[:, kt * 768 + n0 : kt * 768 + n0 + nsz],
                                start=False,
                                stop=(kt == KT - 1),
                            )
                        with nc.allow_low_precision(reason="output in bf16"):
                            nc.scalar.copy(osb[0:tsz, n0 : n0 + nsz], pp[0:tsz, 0:nsz])
                    nc.sync.dma_start(
                        out_d[b * P2 + t0 : b * P2 + t0 + tsz, :], osb[0:tsz, :]
                    )

            def emit_pairblock(pb):
                base = pb * PW
                # x columns for this pair
                xt = xtpool.tile([128, KT * PW], BF16, tag="xt")
                for kt in range(KT):
                    nc.sync.dma_start(
                        xt[:, kt * PW : (kt + 1) * PW],
                        xT_d[:, kt * T2 + base : kt * T2 + base + PW],
                    )
                # stage 1: qkT for the pair: [ch-tile, (mt, 400)]
                qkt = qkpool.tile([128, QKM * PW], BF16, tag="qk")
                for mt in range(QKM):
                    ps = psA.tile([128, 512], F32, tag="a")
                    for kt in range(KT):
                        nc.tensor.matmul(
                            ps[:, 0:PW],
                            wqk[:, kt * 1536 + mt * 128 : kt * 1536 + (mt + 1) * 128],
                            xt[:, kt * PW : (kt + 1) * PW],
                            start=(kt == 0),
                            stop=(kt == KT - 1),
                        )
                    nc.scalar.activation(
                        qkt[:, mt * PW : (mt + 1) * PW],
                        ps[:, 0:PW],
                        AF.Identity,
                        bias=bqk[:, mt : mt + 1],
                        scale=1.0,
                    )
                # stage 2: v for the pair, natural layout per (b2, c)
                vt = vpool.tile([100, 4 * 768], BF16, tag="vt")
                for b2 in range(2):
                    for c in range(2):
                        pbase = b2 * P2 + c * 100
                        for (n0, nsz) in [(0, 512), (512, 256)]:
                            ps = psA.tile([128, 512], F32, tag="a")
                            nc.tensor.matmul(
                                ps[0:100, 0:nsz],
                                ones_row[0:1, 0:100],
                                bv[:, n0 : n0 + nsz],
                                start=True,
                                stop=False,
                            )
                            for kt in range(KT):
                                nc.tensor.matmul(
                                    ps[0:100, 0:nsz],
                                    xt[:, kt * PW + pbase : kt * PW + pbase + 100],
                                    wv[:, kt * 768 + n0 : kt * 768 + n0 + nsz],
                                    start=False,
                                    stop=(kt == KT - 1),
                                )
                            nc.scalar.copy(
                                vt[0:100, (b2 * 2 + c) * 768 + n0 : (b2 * 2 + c) * 768 + n0 + nsz],
                                ps[0:100, 0:nsz],
                            )
                # attention per batch of the pair
                for b2 in range(2):
                    b = pb * 2 + b2
                    expAll = exppool.tile([100, 2 * H * P2], BF16)
                    emit_front(b, b2, expAll, qkt)
                    a2 = emit_mix(expAll)
                    emit_back(b, b2, a2, vt)

            for pb in range(BL // 2):
                emit_pairblock(pb)

    nc.compile()
    return nc


def _tile6(a, width):
    """[768, M] -> [128, 6*M] (K-tile-major host layout)."""
    assert a.shape == (768, width)
    return np.ascontiguousarray(
        a.reshape(KT, 128, width).transpose(1, 0, 2).reshape(128, KT * width)
    )


def _to_bf16(a):
    return np.asarray(a, dtype=np.float32).astype(ml_dtypes.bfloat16)


def _posmaps():
    """token m -> padded position p, and p -> m (or -1 for dummies)."""
    pos_of_tok = np.empty(N, np.int64)
    for m in range(N):
        c = 0 if m < 100 else 1
        mm = m - c * 100
        g, ml = mm // 10, mm % 10
        pos_of_tok[m] = c * 100 + ml * 10 + g
    tok_of_pos = np.full(P2, -1, np.int64)
    tok_of_pos[pos_of_tok] = np.arange(N)
    return pos_of_tok, tok_of_pos


_POS_OF_TOK, _TOK_OF_POS = _posmaps()


def _preprocess(inputs):
    x = np.asarray(inputs["x"], np.float32)
    qkv_w = np.asarray(inputs["qkv_w"], np.float32)
    q_bias = np.asarray(inputs["q_bias"], np.float32)
    v_bias = np.asarray(inputs["v_bias"], np.float32)
    sq = np.asarray(inputs["ssf_scale_qkv"], np.float32)
    tq = np.asarray(inputs["ssf_shift_qkv"], np.float32)
    rbt = np.asarray(inputs["rel_bias_table"], np.float32)
    coeff = np.asarray(inputs["bases_coeff"], np.float32)
    proj_w = np.asarray(inputs["proj_w"], np.float32)
    proj_b = np.asarray(inputs["proj_b"], np.float32)
    sp = np.asarray(inputs["ssf_scale_proj"], np.float32)
    tp = np.asarray(inputs["ssf_shift_proj"], np.float32)
    rel_index = np.asarray(inputs["rel_index"], np.int64)

    qkv_bias = np.concatenate([q_bias, np.zeros_like(q_bias), v_bias])
    w_eff = (qkv_w * sq[:, None]).copy()
    b_eff = (qkv_bias * sq + tq).copy()
    w_eff[0:768] *= SCALE
    b_eff[0:768] *= SCALE

    wqk = _tile6(np.ascontiguousarray(w_eff[0:1536].T), 1536)
    wvt = _tile6(np.ascontiguousarray(w_eff[1536:].T), 768)
    wp_eff = proj_w * sp[:, None]
    bp_eff = proj_b * sp + tp
    wpt = _tile6(np.ascontiguousarray(wp_eff.T), 768)

    bqk_sb = np.ascontiguousarray(b_eff[0:1536].reshape(QKM, 128).T).astype(np.float32)

    # rel bias in permuted+padded coordinates:
    # relb[p, (h*2+c)*P2 + n] = table[rel_index[qtok(n), ktok(c,p)], h]
    # dummy keys get DUMMY_BIAS, dummy queries 0.
    gathered = rbt[rel_index]                      # [query-tok, key-tok, H]
    relb4 = np.zeros((100, H, 2, P2), np.float32)
    q_valid = _TOK_OF_POS >= 0                     # [P2]
    qtok = np.where(q_valid, _TOK_OF_POS, 0)
    for c in range(2):
        ktok_pos = _TOK_OF_POS[c * 100 : (c + 1) * 100]   # [100]
        k_valid = ktok_pos >= 0
        ktok = np.where(k_valid, ktok_pos, 0)
        blk = gathered[qtok[None, :], ktok[:, None], :]   # [100, P2, H]
        blk = blk.transpose(0, 2, 1)                      # [100, H, P2]
        blk = np.where(q_valid[None, None, :], blk, 0.0)
        blk = np.where(k_valid[:, None, None], blk, DUMMY_BIAS)
        relb4[:, :, c, :] = blk
    # upload exp(bias): the kernel multiplies exp(scores) by this instead
    # of adding the bias before the exp (dummy keys -> exp(-40) ~ 0).
    relb = np.exp(relb4.reshape(100, 2 * H * P2))

    # mix = coeff^T * 1.0 + I ; mixblk[wgi*12+h, wgi'*12+k] = d(wgi,wgi')mix[h,k]
    mix = coeff.T + np.eye(H, dtype=np.float32)
    mixblk = np.kron(np.eye(10, dtype=np.float32), mix)
    bv_row = b_eff[1536:].reshape(1, 768)
    bp_row = bp_eff.reshape(1, 768)

    common = {
        "wqk": _to_bf16(wqk),
        "wv": _to_bf16(wvt),
        "wp": _to_bf16(wpt),
        "relb": _to_bf16(relb),
        "mixblk": _to_bf16(mixblk),
        "bqk": bqk_sb,
        "bv": _to_bf16(bv_row),
        "bp": _to_bf16(bp_row),
    }
    in_maps = []
    for ci in range(NCORES):
        xs = x[ci * BL : (ci + 1) * BL]             # [BL, N, C]
        xp = np.zeros((BL, P2, C), np.float32)
        xp[:, _POS_OF_TOK, :] = xs
        xt = xp.reshape(BL * P2, C).T               # [C, T2]
        m = dict(common)
        m["xT"] = _to_bf16(_tile6(np.ascontiguousarray(xt), T2))
        in_maps.append(m)
    return in_maps


def _get_compiled():
    if "nc" not in _COMPILED:
        _COMPILED["nc"] = _build_graph()
    return _COMPILED["nc"]


LAST_EXEC_NS = None
LAST_RESULTS = None


def _ensure_ntff_hook():
    """The agent image's antenv package lacks axon_hooks; synthesize it so
    run_bass_kernel_spmd(trace=True) can capture NTFF profiles."""
    import types

    if "antenv.axon_hooks" in sys.modules:
        return
    try:
        sys.path.insert(0, "/root/.axon_site")
        from trn_agent_boot.trn_boot import _ntff_profile_via_ctypes

        hook = _ntff_profile_via_ctypes("/opt/axon/libaxon_pjrt.so")
    except Exception:
        hook = None
    mod = types.ModuleType("antenv.axon_hooks")
    _state = {"hook": hook}
    mod.get_axon_ntff_profile_hook = lambda: _state["hook"]
    mod.set_axon_ntff_profile_hook = lambda h: _state.__setitem__("hook", h)
    sys.modules["antenv.axon_hooks"] = mod


def kernel(**inputs) -> np.ndarray:
    global LAST_EXEC_NS, LAST_RESULTS
    nc = _get_compiled()
    in_maps = _preprocess(inputs)
    from concourse.bass_utils import run_bass_kernel_spmd

    trace = os.environ.get("BASS_KERNEL_PROFILE", "0") == "1"
    if trace:
        _ensure_ntff_hook()
    res = run_bass_kernel_spmd(nc, in_maps, core_ids=list(range(NCORES)), trace=trace)
    LAST_EXEC_NS = res.exec_time_ns
    LAST_RESULTS = res
    outs = []
    for i in range(NCORES):
        o = np.asarray(res.results[i]["out"], np.float32).reshape(BL, P2, C)
        outs.append(o[:, _POS_OF_TOK, :])           # drop dummies, un-permute
    return np.concatenate(outs, axis=0).astype(np.float32)


# revision 22
# speedup vs baseline: 1.2115x; 1.2115x over previous
"""Trainium2 Bass kernel for nn_Attention_39608188404100.

Windowed-attention block (ViT-style, N=197 tokens) with SSF affines, relative
position bias, DCF head mixing, and output projection.

Strategy: pure data-parallel over batch across 8 NeuronCores (B=64 -> 8/core).
All weights replicated; no collectives. Compute in bf16 on the TensorEngine
(fp32 PSUM accumulation). fp8 was tried and rejected: each e4m3 quantization
point adds ~3.6% RMS error, far over the 2% tolerance.

Per core (BL=8 batches): each batch's 197 tokens are padded to 200 positions
and PERMUTED on host: position p = c*100 + ml*10 + g holds token
m = c*100 + g*10 + ml (c = chunk, 2x100). The 3 dummy positions per batch get
zero x-columns and a -40 relative-bias on their key rows; dummy query columns
are dropped on host after download.

Pipeline: dense QKV+V phase first (TensorEngine at ~95%), then a per-batch
attention loop emitted in an explicit 3-stage software-pipelined order
(front(b+2), back(b), mix(b+1)) so the scheduler overlaps three batches'
chains: scores/exp/softmax-epilogue (front), keypos<->(wgi,h) shuffle +
block-diagonal [120x120] DCF mix (mix), AV + projection (back). Shuffles are
10 direct SBUF->SBUF DMAs per direction per batch (plain-slice APs, 800B
descriptors).

Softmax denominator: ones-column matmul into PSUM, in-place fp32
reciprocal_approx_fast on the PSUM row, bf16 row, DRAM-bounce broadcast DMA
to 100 partitions, two in-place normalize multiplies.
"""
import os
import sys

sys.path.insert(0, "/opt/trn_rl_repo")

import numpy as np
import ml_dtypes

import concourse.bass as bass
import concourse.tile as tile
from concourse import bacc, mybir

BF16 = mybir.dt.bfloat16
F32 = mybir.dt.float32
AF = mybir.ActivationFunctionType
ALU = mybir.AluOpType

B, N, C, H = 64, 197, 768, 12
NCORES = 8
BL = B // NCORES          # 8 batches per core
P2 = 200                  # padded positions per batch
T2 = BL * P2              # 1600 positions per core
PW = 2 * P2               # pair width (400 token columns)
SCALE = (C // H) ** -0.5
KT = 6                    # contraction tiles of 128 over C=768
QKM = 12                  # 128-wide M tiles over 1536 q/k channels
DUMMY_BIAS = -40.0
DEN_CH = 480              # denominator column chunk (5 x 480 = H*P2)

_COMPILED = {}


def _build_graph():
    nc = bacc.Bacc(
        "TRN2", target_bir_lowering=False, debug=False,
        detect_race_conditions=False,
    )

    xT_d = nc.dram_tensor("xT", [128, KT * T2], BF16, kind="ExternalInput")
    wqk_d = nc.dram_tensor("wqk", [128, KT * 1536], BF16, kind="ExternalInput")
    wv_d = nc.dram_tensor("wv", [128, KT * 768], BF16, kind="ExternalInput")
    wp_d = nc.dram_tensor("wp", [128, KT * 768], BF16, kind="ExternalInput")
    relb_d = nc.dram_tensor("relb", [100, 2 * H * P2], BF16, kind="ExternalInput")
    mix_d = nc.dram_tensor("mixblk", [120, 120], BF16, kind="ExternalInput")
    bqk_d = nc.dram_tensor("bqk", [128, QKM], F32, kind="ExternalInput")
    bv_d = nc.dram_tensor("bv", [1, 768], BF16, kind="ExternalInput")
    bp_d = nc.dram_tensor("bp", [1, 768], BF16, kind="ExternalInput")
    out_d = nc.dram_tensor("out", [T2, 768], BF16, kind="ExternalOutput")

    with tile.TileContext(nc) as tc:
        with (
            tc.tile_pool(name="const", bufs=1) as cpool,
            tc.tile_pool(name="dram", bufs=2, space=bass.MemorySpace.DRAM) as drpool,
            tc.tile_pool(name="psA", bufs=2, space=bass.MemorySpace.PSUM) as psA,
            tc.tile_pool(name="psS", bufs=2, space=bass.MemorySpace.PSUM) as psS,
            tc.tile_pool(name="psD", bufs=2, space=bass.MemorySpace.PSUM) as psD,
            tc.tile_pool(name="psMV", bufs=2, space=bass.MemorySpace.PSUM) as psMV,
        ):
            # ---- persistent constants / weights ----
            wp = cpool.tile([128, KT * 768], BF16)
            relb = cpool.tile([100, 2 * H * P2], BF16)
            mixblk = cpool.tile([120, 120], BF16)
            bqk = cpool.tile([128, QKM], F32)
            bv = cpool.tile([1, 768], BF16)
            bp = cpool.tile([1, 768], BF16)
            ones_col = cpool.tile([128, 1], BF16)   # lhsT for denominator
            ones_row = cpool.tile([1, 128], BF16)   # lhsT for rank-1 bias
            nc.sync.dma_start(wp[:], wp_d[:])
            nc.sync.dma_start(relb[:], relb_d[:])
            nc.sync.dma_start(mixblk[:], mix_d[:])
            nc.sync.dma_start(bqk[:], bqk_d[:])
            nc.sync.dma_start(bv[:], bv_d[:])
            nc.sync.dma_start(bp[:], bp_d[:])
            nc.vector.memset(ones_col[:], 1.0)
            nc.vector.memset(ones_row[:], 1.0)

            qk_sb = cpool.tile([128, QKM * T2], BF16)      # qkT: [ch-tile, pos]
            v_sb = cpool.tile([100, 2 * BL * 768], BF16)   # v: [pos, (b,c)*768+ch]

            def emit_stage12(epool):
                xT = epool.tile([128, KT * T2], BF16)
                wqk = epool.tile([128, KT * 1536], BF16)
                wv = epool.tile([128, KT * 768], BF16)
                for kt in range(KT):
                    nc.sync.dma_start(
                        wqk[:, kt * 1536 : (kt + 1) * 1536],
                        wqk_d[:, kt * 1536 : (kt + 1) * 1536],
                    )
                for half in range(2):
                    h0 = half * (T2 // 2)
                    for kt in range(KT):
                        nc.sync.dma_start(
                            xT[:, kt * T2 + h0 : kt * T2 + h0 + T2 // 2],
                            xT_d[:, kt * T2 + h0 : kt * T2 + h0 + T2 // 2],
                        )
                nc.sync.dma_start(wv[:], wv_d[:])

                for mt in range(QKM):
                    for n0 in range(0, T2, 400):
                        ps = psA.tile([128, 512], F32, tag="a")
                        for kt in range(KT):
                            nc.tensor.matmul(
                                ps[:, 0:400],
                                wqk[:, kt * 1536 + mt * 128 : kt * 1536 + (mt + 1) * 128],
                                xT[:, kt * T2 + n0 : kt * T2 + n0 + 400],
                                start=(kt == 0),
                                stop=(kt == KT - 1),
                            )
                        nc.scalar.activation(
                            qk_sb[:, mt * T2 + n0 : mt * T2 + n0 + 400],
                            ps[:, 0:400],
                            AF.Identity,
                            bias=bqk[:, mt : mt + 1],
                            scale=1.0,
                        )

                for b in range(BL):
                    for c in range(2):
                        base = b * P2 + c * 100
                        for (n0, nsz) in [(0, 512), (512, 256)]:
                            ps = psA.tile([128, 512], F32, tag="a")
                            nc.tensor.matmul(
                                ps[0:100, 0:nsz],
                                ones_row[0:1, 0:100],
                                bv[:, n0 : n0 + nsz],
                                start=True,
                                stop=False,
                            )
                            for kt in range(KT):
                                nc.tensor.matmul(
                                    ps[0:100, 0:nsz],
                                    xT[:, kt * T2 + base : kt * T2 + base + 100],
                                    wv[:, kt * 768 + n0 : kt * 768 + n0 + nsz],
                                    start=False,
                                    stop=(kt == KT - 1),
                                )
                            nc.scalar.copy(
                                v_sb[0:100, (b * 2 + c) * 768 + n0 : (b * 2 + c) * 768 + n0 + nsz],
                                ps[0:100, 0:nsz],
                            )

            def emit_front(b, expAll):
                """Scores, exp, softmax epilogue, in-place normalize."""
                eP = expAll[:].rearrange("p (h two n) -> p h two n",
                                         h=H, two=2, n=P2)
                for h in range(H):
                    prow = (h % 2) * 64
                    qoff = (h // 2) * T2 + b * P2
                    koff = (6 + h // 2) * T2 + b * P2

                    ps1 = psS.tile([128, 512], F32, tag="s")
                    nc.tensor.matmul(
                        ps1[0:100, 0:P2],
                        qk_sb[prow : prow + 64, koff : koff + 100],
                        qk_sb[prow : prow + 64, qoff : qoff + P2],
                        start=True, stop=True,
                    )
                    nc.tensor.matmul(
                        ps1[0:100, P2 : 2 * P2],
                        qk_sb[prow : prow + 64, koff + 100 : koff + 200],
                        qk_sb[prow : prow + 64, qoff : qoff + P2],
                        start=True, stop=True,
                    )
                    nc.scalar.activation(eP[0:100, h], ps1[0:100, 0 : 2 * P2],
                                         AF.Exp)

                # softmax epilogue: exp(rel-bias) multiply + chunk-sum in two
                # head-halves, ones-matmul denominator per 480-col chunk,
                # in-place fp32 fast reciprocal in PSUM, bf16 row, DRAM-bounce
                # broadcast, two in-place normalize multiplies.
                ev = eP[0:100]                              # [100, h, two, n]
                rv = relb[0:100, :].rearrange("p (h c n) -> p h c n", h=H, c=2, n=P2)
                denb = smallpool.tile([100, H * P2], BF16, tag="denb")
                dv = denb[:].rearrange("p (h n) -> p h n", h=H)
                for h0 in (0, 6):
                    nc.vector.tensor_tensor(ev[:, h0 : h0 + 6], ev[:, h0 : h0 + 6],
                                            rv[:, h0 : h0 + 6], ALU.mult)
                    nc.vector.tensor_tensor(dv[:, h0 : h0 + 6],
                                            ev[:, h0 : h0 + 6, 0, :],
                                            ev[:, h0 : h0 + 6, 1, :], ALU.add)
                denr = dbpool.tile([1, H * P2], BF16, tag="denr")
                for o in range(0, H * P2, DEN_CH):
                    psd = psD.tile([128, 512], F32, tag="d")
                    nc.tensor.matmul(psd[0:1, 0:DEN_CH], ones_col[0:100, 0:1],
                                     denb[0:100, o : o + DEN_CH],
                                     start=True, stop=True)
                    nc.vector.reciprocal_approx_fast(psd[0:1, 0:DEN_CH],
                                                     psd[0:1, 0:DEN_CH])
                    with nc.allow_low_precision(reason="softmax recip in bf16"):
                        nc.vector.tensor_copy(denr[0:1, o : o + DEN_CH],
                                              psd[0:1, 0:DEN_CH])
                ddr = drpool.tile([1, H * P2], BF16, tag="ddr")
                nc.gpsimd.dma_start(ddr[:], denr[:])
                denbc = dbpool.tile([100, H * P2], BF16, tag="denbc")
                nc.gpsimd.dma_start(denbc[:], ddr[:].to_broadcast([100, H * P2]))
                dvb = denbc[:].rearrange("p (h n) -> p h n", h=H)
                nc.vector.tensor_tensor(ev[:, :, 0, :], ev[:, :, 0, :], dvb, ALU.mult)
                nc.vector.tensor_tensor(ev[:, :, 1, :], ev[:, :, 1, :], dvb, ALU.mult)

            def emit_mix(expAll):
                """keypos->(wgi,h) shuffle, block-diag mix, shuffle back.
                mxin[wgi*12+h, j*400+(two n)] = expAll[j*10+wgi, h*400+(two n)]"""
                mxin = mxpool.tile([120, 10 * PW], BF16, tag="mxin")
                for j in range(10):
                    eng = nc.sync if j % 2 == 0 else nc.gpsimd
                    eng.dma_start(mxin[:, j * 400 : (j + 1) * 400],
                                  expAll[j * 10 : (j + 1) * 10, :])
                mxo = mopool.tile([120, 10 * PW], BF16, tag="mxout")
                for o in range(0, 4000, 500):
                    psm = psMV.tile([128, 512], F32, tag="mv")
                    nc.tensor.matmul(
                        psm[0:120, 0:500], mixblk[:],
                        mxin[:, o : o + 500],
                        start=True, stop=True,
                    )
                    nc.scalar.copy(mxo[:, o : o + 500], psm[0:120, 0:500])
                a2 = a2pool.tile([100, 2 * H * P2], BF16)  # [keypos, (k,two,n)]
                for j in range(10):
                    eng = nc.sync if j % 2 == 0 else nc.gpsimd
                    eng.dma_start(a2[j * 10 : (j + 1) * 10, :],
                                  mxo[:, j * 400 : (j + 1) * 400])
                return a2

            def emit_back(b, a2):
                """AV + projection for one batch."""
                a2v = a2[:].rearrange("p (k two n) -> p k two n",
                                      k=H, two=2, n=P2)
                aoT = aopool.tile([128, KT * P2], BF16, tag="ao")
                for tt in range(H // 4):        # 3 psum tiles, 2 head-pairs each
                    pv = psMV.tile([128, 512], F32, tag="mv")
                    for jj2 in range(2):
                        jj = tt * 2 + jj2
                        for sub in range(2):
                            k = 2 * jj + sub
                            rows = pv[sub * 64 : sub * 64 + 64,
                                      jj2 * P2 : (jj2 + 1) * P2]
                            for c in range(2):
                                nc.tensor.matmul(
                                    rows,
                                    v_sb[0:100, (b * 2 + c) * 768 + k * 64 : (b * 2 + c) * 768 + (k + 1) * 64],
                                    a2v[0:100, k, c],
                                    start=(c == 0),
                                    stop=(c == 1),
                                    tile_position=(0, sub * 64),
                                )
                    nc.scalar.copy(aoT[:, tt * 2 * P2 : (tt + 1) * 2 * P2],
                                   pv[:, 0 : 2 * P2])

                for (t0, tsz) in [(0, 128), (128, 72)]:
                    osb = opool.tile([128, 768], BF16, tag="osb")
                    for (n0, nsz) in [(0, 512), (512, 256)]:
                        pp = psA.tile([128, 512], F32, tag="a")
                        nc.tensor.matmul(
                            pp[0:tsz, 0:nsz],
                            ones_row[0:1, 0:tsz],
                            bp[:, n0 : n0 + nsz],
                            start=True, stop=False,
                        )
                        for kt in range(KT):
                            nc.tensor.matmul(
                                pp[0:tsz, 0:nsz],
                                aoT[:, kt * P2 + t0 : kt * P2 + t0 + tsz],
                                wp[:, kt * 768 + n0 : kt * 768 + n0 + nsz],
                                start=False,
                                stop=(kt == KT - 1),
                            )
                        with nc.allow_low_precision(reason="output in bf16"):
                            nc.scalar.copy(osb[0:tsz, n0 : n0 + nsz], pp[0:tsz, 0:nsz])
                    nc.sync.dma_start(
                        out_d[b * P2 + t0 : b * P2 + t0 + tsz, :], osb[0:tsz, :]
                    )

            # 3-stage software-pipelined emission:
            #   f0 | f1 m0 | f2 b0 m1 | f3 b1 m2 | ... | b6 m7 | b7
            def emit_attention():
                exps = {}
                a2s = {}
                for b in range(BL):
                    expAll = exppool.tile([100, 2 * H * P2], BF16)
                    exps[b] = expAll
                    emit_front(b, expAll)
                    if b >= 2:
                        emit_back(b - 2, a2s.pop(b - 2))
                    if b >= 1:
                        a2s[b - 1] = emit_mix(exps.pop(b - 1))
                emit_back(BL - 2, a2s.pop(BL - 2))
                a2s[BL - 1] = emit_mix(exps.pop(BL - 1))
                emit_back(BL - 1, a2s.pop(BL - 1))

            exppool = smallpool = a2pool = mxpool = mopool = None
            aopool = dbpool = opool = None

            with tc.tile_pool(name="early", bufs=1) as epool:
                emit_stage12(epool)

            with (
                tc.tile_pool(name="exp", bufs=3) as exppool,
                tc.tile_pool(name="small", bufs=2) as smallpool,
                tc.tile_pool(name="a2", bufs=2) as a2pool,
                tc.tile_pool(name="mxin", bufs=2) as mxpool,
                tc.tile_pool(name="mxout", bufs=2) as mopool,
                tc.tile_pool(name="ao", bufs=2) as aopool,
                tc.tile_pool(name="denb2", bufs=2) as dbpool,
                tc.tile_pool(name="osb", bufs=2) as opool,
            ):
                emit_attention()

    nc.compile()
    return nc


def _tile6(a, width):
    """[768, M] -> [128, 6*M] (K-tile-major host layout)."""
    assert a.shape == (768, width)
    return np.ascontiguousarray(
        a.reshape(KT, 128, width).transpose(1, 0, 2).reshape(128, KT * width)
    )


def _to_bf16(a):
    return np.asarray(a, dtype=np.float32).astype(ml_dtypes.bfloat16)


def _posmaps():
    """token m -> padded position p, and p -> m (or -1 for dummies)."""
    pos_of_tok = np.empty(N, np.int64)
    for m in range(N):
        c = 0 if m < 100 else 1
        mm = m - c * 100
        g, ml = mm // 10, mm % 10
        pos_of_tok[m] = c * 100 + ml * 10 + g
    tok_of_pos = np.full(P2, -1, np.int64)
    tok_of_pos[pos_of_tok] = np.arange(N)
    return pos_of_tok, tok_of_pos


_POS_OF_TOK, _TOK_OF_POS = _posmaps()


def _preprocess(inputs):
    x = np.asarray(inputs["x"], np.float32)
    qkv_w = np.asarray(inputs["qkv_w"], np.float32)
    q_bias = np.asarray(inputs["q_bias"], np.float32)
    v_bias = np.asarray(inputs["v_bias"], np.float32)
    sq = np.asarray(inputs["ssf_scale_qkv"], np.float32)
    tq = np.asarray(inputs["ssf_shift_qkv"], np.float32)
    rbt = np.asarray(inputs["rel_bias_table"], np.float32)
    coeff = np.asarray(inputs["bases_coeff"], np.float32)
    proj_w = np.asarray(inputs["proj_w"], np.float32)
    proj_b = np.asarray(inputs["proj_b"], np.float32)
    sp = np.asarray(inputs["ssf_scale_proj"], np.float32)
    tp = np.asarray(inputs["ssf_shift_proj"], np.float32)
    rel_index = np.asarray(inputs["rel_index"], np.int64)

    qkv_bias = np.concatenate([q_bias, np.zeros_like(q_bias), v_bias])
    w_eff = (qkv_w * sq[:, None]).copy()
    b_eff = (qkv_bias * sq + tq).copy()
    w_eff[0:768] *= SCALE
    b_eff[0:768] *= SCALE

    wqk = _tile6(np.ascontiguousarray(w_eff[0:1536].T), 1536)
    wvt = _tile6(np.ascontiguousarray(w_eff[1536:].T), 768)
    wp_eff = proj_w * sp[:, None]
    bp_eff = proj_b * sp + tp
    wpt = _tile6(np.ascontiguousarray(wp_eff.T), 768)

    bqk_sb = np.ascontiguousarray(b_eff[0:1536].reshape(QKM, 128).T).astype(np.float32)

    # rel bias in permuted+padded coordinates:
    # relb[p, (h*2+c)*P2 + n] = table[rel_index[qtok(n), ktok(c,p)], h]
    # dummy keys get DUMMY_BIAS, dummy queries 0.
    gathered = rbt[rel_index]                      # [query-tok, key-tok, H]
    relb4 = np.zeros((100, H, 2, P2), np.float32)
    q_valid = _TOK_OF_POS >= 0                     # [P2]
    qtok = np.where(q_valid, _TOK_OF_POS, 0)
    for c in range(2):
        ktok_pos = _TOK_OF_POS[c * 100 : (c + 1) * 100]   # [100]
        k_valid = ktok_pos >= 0
        ktok = np.where(k_valid, ktok_pos, 0)
        blk = gathered[qtok[None, :], ktok[:, None], :]   # [100, P2, H]
        blk = blk.transpose(0, 2, 1)                      # [100, H, P2]
        blk = np.where(q_valid[None, None, :], blk, 0.0)
        blk = np.where(k_valid[:, None, None], blk, DUMMY_BIAS)
        relb4[:, :, c, :] = blk
    # upload exp(bias): the kernel multiplies exp(scores) by this instead
    # of adding the bias before the exp (dummy keys -> exp(-40) ~ 0).
    relb = np.exp(relb4.reshape(100, 2 * H * P2))

    # mix = coeff^T * 1.0 + I ; mixblk[wgi*12+h, wgi'*12+k] = d(wgi,wgi')mix[h,k]
    mix = coeff.T + np.eye(H, dtype=np.float32)
    mixblk = np.kron(np.eye(10, dtype=np.float32), mix)
    bv_row = b_eff[1536:].reshape(1, 768)
    bp_row = bp_eff.reshape(1, 768)

    common = {
        "wqk": _to_bf16(wqk),
        "wv": _to_bf16(wvt),
        "wp": _to_bf16(wpt),
        "relb": _to_bf16(relb),
        "mixblk": _to_bf16(mixblk),
        "bqk": bqk_sb,
        "bv": _to_bf16(bv_row),
        "bp": _to_bf16(bp_row),
    }
    in_maps = []
    for ci in range(NCORES):
        xs = x[ci * BL : (ci + 1) * BL]             # [BL, N, C]
        xp = np.zeros((BL, P2, C), np.float32)
        xp[:, _POS_OF_TOK, :] = xs
        xt = xp.reshape(BL * P2, C).T               # [C, T2]
        m = dict(common)
        m["xT"] = _to_bf16(_tile6(np.ascontiguousarray(xt), T2))
        in_maps.append(m)
    return in_maps


def _get_compiled():
    if "nc" not in _COMPILED:
        _COMPILED["nc"] = _build_graph()
    return _COMPILED["nc"]


LAST_EXEC_NS = None
LAST_RESULTS = None


def _ensure_ntff_hook():
    """The agent image's antenv package lacks axon_hooks; synthesize it so
    run_bass_kernel_spmd(trace=True) can capture NTFF profiles."""
    import types

    if "antenv.axon_hooks" in sys.modules:
        return
    try:
        sys.path.insert(0, "/root/.axon_site")
        from trn_agent_boot.trn_boot import _ntff_profile_via_ctypes

        hook = _ntff_profile_via_ctypes("/opt/axon/libaxon_pjrt.so")
    except Exception:
        hook = None
    mod = types.ModuleType("antenv.axon_hooks")
    _state = {"hook": hook}
    mod.get_axon_ntff_profile_hook = lambda: _state["hook"]
    mod.set_axon_ntff_profile_hook = lambda h: _state.__setitem__("hook", h)
    sys.modules["antenv.axon_hooks"] = mod


def kernel(**inputs) -> np.ndarray:
    global LAST_EXEC_NS, LAST_RESULTS
    nc = _get_compiled()
    in_maps = _preprocess(inputs)
    from concourse.bass_utils import run_bass_kernel_spmd

    trace = os.environ.get("BASS_KERNEL_PROFILE", "0") == "1"
    if trace:
        _ensure_ntff_hook()
    res = run_bass_kernel_spmd(nc, in_maps, core_ids=list(range(NCORES)), trace=trace)
    LAST_EXEC_NS = res.exec_time_ns
    LAST_RESULTS = res
    outs = []
    for i in range(NCORES):
        o = np.asarray(res.results[i]["out"], np.float32).reshape(BL, P2, C)
        outs.append(o[:, _POS_OF_TOK, :])           # drop dummies, un-permute
    return np.concatenate(outs, axis=0).astype(np.float32)


# revision 23
# speedup vs baseline: 1.2273x; 1.0130x over previous
"""Trainium2 Bass kernel for nn_Attention_39608188404100.

Windowed-attention block (ViT-style, N=197 tokens) with SSF affines, relative
position bias, DCF head mixing, and output projection.

Strategy: pure data-parallel over batch across 8 NeuronCores (B=64 -> 8/core).
All weights replicated; no collectives. Compute in bf16 on the TensorEngine
(fp32 PSUM accumulation). fp8 was tried and rejected: each e4m3 quantization
point adds ~3.6% RMS error, far over the 2% tolerance.

Per core (BL=8 batches): each batch's 197 tokens are padded to 200 positions
and PERMUTED on host: position p = c*100 + ml*10 + g holds token
m = c*100 + g*10 + ml (c = chunk, 2x100). The 3 dummy positions per batch get
zero x-columns and a -40 relative-bias on their key rows; dummy query columns
are dropped on host after download.

Pipeline: dense QKV+V phase first (TensorEngine at ~95%), then a per-batch
attention loop emitted in an explicit 3-stage software-pipelined order
(front(b+2), back(b), mix(b+1)) so the scheduler overlaps three batches'
chains: scores/exp/softmax-epilogue (front), keypos<->(wgi,h) shuffle +
block-diagonal [120x120] DCF mix (mix), AV + projection (back). Shuffles are
10 direct SBUF->SBUF DMAs per direction per batch (plain-slice APs, 800B
descriptors).

Softmax denominator: ones-column matmul into PSUM, in-place fp32
reciprocal_approx_fast on the PSUM row, bf16 row, DRAM-bounce broadcast DMA
to 100 partitions, two in-place normalize multiplies.
"""
import os
import sys

sys.path.insert(0, "/opt/trn_rl_repo")

import numpy as np
import ml_dtypes

import concourse.bass as bass
import concourse.tile as tile
from concourse import bacc, mybir

BF16 = mybir.dt.bfloat16
F32 = mybir.dt.float32
AF = mybir.ActivationFunctionType
ALU = mybir.AluOpType

B, N, C, H = 64, 197, 768, 12
NCORES = 8
BL = B // NCORES          # 8 batches per core
P2 = 200                  # padded positions per batch
T2 = BL * P2              # 1600 positions per core
PW = 2 * P2               # pair width (400 token columns)
SCALE = (C // H) ** -0.5
KT = 6                    # contraction tiles of 128 over C=768
QKM = 12                  # 128-wide M tiles over 1536 q/k channels
DUMMY_BIAS = -40.0
DEN_CH = 480              # denominator column chunk (5 x 480 = H*P2)

_COMPILED = {}


def _build_graph():
    nc = bacc.Bacc(
        "TRN2", target_bir_lowering=False, debug=False,
        detect_race_conditions=False,
    )

    xT_d = nc.dram_tensor("xT", [128, KT * T2], BF16, kind="ExternalInput")
    wqk_d = nc.dram_tensor("wqk", [128, KT * 1536], BF16, kind="ExternalInput")
    wv_d = nc.dram_tensor("wv", [128, KT * 768], BF16, kind="ExternalInput")
    wp_d = nc.dram_tensor("wp", [128, KT * 768], BF16, kind="ExternalInput")
    relb_d = nc.dram_tensor("relb", [100, 2 * H * P2], BF16, kind="ExternalInput")
    mix_d = nc.dram_tensor("mixblk", [120, 120], BF16, kind="ExternalInput")
    bqk_d = nc.dram_tensor("bqk", [128, QKM], F32, kind="ExternalInput")
    bp_d = nc.dram_tensor("bp", [1, 768], BF16, kind="ExternalInput")
    out_d = nc.dram_tensor("out", [T2, 768], BF16, kind="ExternalOutput")

    with tile.TileContext(nc) as tc:
        with (
            tc.tile_pool(name="const", bufs=1) as cpool,
            tc.tile_pool(name="dram", bufs=2, space=bass.MemorySpace.DRAM) as drpool,
            tc.tile_pool(name="psA", bufs=2, space=bass.MemorySpace.PSUM) as psA,
            tc.tile_pool(name="psS", bufs=2, space=bass.MemorySpace.PSUM) as psS,
            tc.tile_pool(name="psD", bufs=1, space=bass.MemorySpace.PSUM) as psD,
            tc.tile_pool(name="psMV", bufs=3, space=bass.MemorySpace.PSUM) as psMV,
        ):
            # ---- persistent constants / weights ----
            wp = cpool.tile([128, KT * 768], BF16)
            relb = cpool.tile([100, 2 * H * P2], BF16)
            mixblk = cpool.tile([120, 120], BF16)
            bqk = cpool.tile([128, QKM], F32)
            bp = cpool.tile([1, 768], BF16)
            ones_col = cpool.tile([128, 1], BF16)   # lhsT for denominator
            ones_row = cpool.tile([1, 128], BF16)   # lhsT for rank-1 bias
            nc.sync.dma_start(wp[:], wp_d[:])
            nc.sync.dma_start(relb[:], relb_d[:])
            nc.sync.dma_start(mixblk[:], mix_d[:])
            nc.sync.dma_start(bqk[:], bqk_d[:])
            nc.sync.dma_start(bp[:], bp_d[:])
            nc.vector.memset(ones_col[:], 1.0)
            nc.vector.memset(ones_row[:], 1.0)

            qk_sb = cpool.tile([128, QKM * T2], BF16)      # qkT: [ch-tile, pos]
            v_sb = cpool.tile([100, 2 * BL * 768], BF16)   # v: [pos, (b,c)*768+ch]

            def emit_stage12(epool):
                xT = epool.tile([128, KT * T2], BF16)
                wqk = epool.tile([128, KT * 1536], BF16)
                wv = epool.tile([128, KT * 768], BF16)
                for kt in range(KT):
                    nc.sync.dma_start(
                        wqk[:, kt * 1536 : (kt + 1) * 1536],
                        wqk_d[:, kt * 1536 : (kt + 1) * 1536],
                    )
                for half in range(2):
                    h0 = half * (T2 // 2)
                    for kt in range(KT):
                        nc.sync.dma_start(
                            xT[:, kt * T2 + h0 : kt * T2 + h0 + T2 // 2],
                            xT_d[:, kt * T2 + h0 : kt * T2 + h0 + T2 // 2],
                        )
                nc.sync.dma_start(wv[:], wv_d[:])

                for mt in range(QKM):
                    for n0 in range(0, T2, 400):
                        ps = psA.tile([128, 512], F32, tag="a")
                        for kt in range(KT):
                            nc.tensor.matmul(
                                ps[:, 0:400],
                                wqk[:, kt * 1536 + mt * 128 : kt * 1536 + (mt + 1) * 128],
                                xT[:, kt * T2 + n0 : kt * T2 + n0 + 400],
                                start=(kt == 0),
                                stop=(kt == KT - 1),
                            )
                        nc.scalar.activation(
                            qk_sb[:, mt * T2 + n0 : mt * T2 + n0 + 400],
                            ps[:, 0:400],
                            AF.Identity,
                            bias=bqk[:, mt : mt + 1],
                            scale=1.0,
                        )

                # v carries NO bias: since softmax rows sum to exactly 1,
                # the v-bias term through the mixed attention is the constant
                # (sum_h mix[h,k]) * bv_k per head -- folded into bp on host.
                for b in range(BL):
                    for c in range(2):
                        base = b * P2 + c * 100
                        for (n0, nsz) in [(0, 512), (512, 256)]:
                            ps = psA.tile([128, 512], F32, tag="a")
                            for kt in range(KT):
                                nc.tensor.matmul(
                                    ps[0:100, 0:nsz],
                                    xT[:, kt * T2 + base : kt * T2 + base + 100],
                                    wv[:, kt * 768 + n0 : kt * 768 + n0 + nsz],
                                    start=(kt == 0),
                                    stop=(kt == KT - 1),
                                )
                            nc.scalar.copy(
                                v_sb[0:100, (b * 2 + c) * 768 + n0 : (b * 2 + c) * 768 + n0 + nsz],
                                ps[0:100, 0:nsz],
                            )

            def emit_front(b, expAll):
                """Scores, exp, softmax epilogue, in-place normalize."""
                eP = expAll[:].rearrange("p (h two n) -> p h two n",
                                         h=H, two=2, n=P2)
                for h in range(H):
                    prow = (h % 2) * 64
                    qoff = (h // 2) * T2 + b * P2
                    koff = (6 + h // 2) * T2 + b * P2

                    ps1 = psS.tile([128, 512], F32, tag="s")
                    nc.tensor.matmul(
                        ps1[0:100, 0:P2],
                        qk_sb[prow : prow + 64, koff : koff + 100],
                        qk_sb[prow : prow + 64, qoff : qoff + P2],
                        start=True, stop=True,
                    )
                    nc.tensor.matmul(
                        ps1[0:100, P2 : 2 * P2],
                        qk_sb[prow : prow + 64, koff + 100 : koff + 200],
                        qk_sb[prow : prow + 64, qoff : qoff + P2],
                        start=True, stop=True,
                    )
                    nc.scalar.activation(eP[0:100, h], ps1[0:100, 0 : 2 * P2],
                                         AF.Exp)

                # softmax epilogue: exp(rel-bias) multiply + chunk-sum in two
                # head-halves, ones-matmul denominator per 480-col chunk,
                # in-place fp32 fast reciprocal in PSUM, bf16 row, DRAM-bounce
                # broadcast, two in-place normalize multiplies.
                ev = eP[0:100]                              # [100, h, two, n]
                rv = relb[0:100, :].rearrange("p (h c n) -> p h c n", h=H, c=2, n=P2)
                denb = smallpool.tile([100, H * P2], BF16, tag="denb")
                dv = denb[:].rearrange("p (h n) -> p h n", h=H)
                for h0 in (0, 6):
                    nc.vector.tensor_tensor(ev[:, h0 : h0 + 6], ev[:, h0 : h0 + 6],
                                            rv[:, h0 : h0 + 6], ALU.mult)
                    nc.vector.tensor_tensor(dv[:, h0 : h0 + 6],
                                            ev[:, h0 : h0 + 6, 0, :],
                                            ev[:, h0 : h0 + 6, 1, :], ALU.add)
                denr = dbpool.tile([1, H * P2], BF16, tag="denr")
                for o in range(0, H * P2, DEN_CH):
                    psd = psD.tile([128, 512], F32, tag="d")
                    nc.tensor.matmul(psd[0:1, 0:DEN_CH], ones_col[0:100, 0:1],
                                     denb[0:100, o : o + DEN_CH],
                                     start=True, stop=True)
                    nc.vector.reciprocal_approx_fast(psd[0:1, 0:DEN_CH],
                                                     psd[0:1, 0:DEN_CH])
                    with nc.allow_low_precision(reason="softmax recip in bf16"):
                        nc.vector.tensor_copy(denr[0:1, o : o + DEN_CH],
                                              psd[0:1, 0:DEN_CH])
                ddr = drpool.tile([1, H * P2], BF16, tag="ddr")
                nc.gpsimd.dma_start(ddr[:], denr[:])
                denbc = dbpool.tile([100, H * P2], BF16, tag="denbc")
                nc.gpsimd.dma_start(denbc[:], ddr[:].to_broadcast([100, H * P2]))
                dvb = denbc[:].rearrange("p (h n) -> p h n", h=H)
                nc.vector.tensor_tensor(ev[:, :, 0, :], ev[:, :, 0, :], dvb, ALU.mult)
                nc.vector.tensor_tensor(ev[:, :, 1, :], ev[:, :, 1, :], dvb, ALU.mult)

            def emit_mix(expAll):
                """keypos->(wgi,h) shuffle, block-diag mix, shuffle back.
                mxin[wgi*12+h, j*400+(two n)] = expAll[j*10+wgi, h*400+(two n)]"""
                mxin = mxpool.tile([120, 10 * PW], BF16, tag="mxin")
                for j in range(10):
                    eng = nc.sync if j % 2 == 0 else nc.gpsimd
                    eng.dma_start(mxin[:, j * 400 : (j + 1) * 400],
                                  expAll[j * 10 : (j + 1) * 10, :])
                mxo = mopool.tile([120, 10 * PW], BF16, tag="mxout")
                for o in range(0, 4000, 500):
                    psm = psMV.tile([128, 512], F32, tag="mv")
                    nc.tensor.matmul(
                        psm[0:120, 0:500], mixblk[:],
                        mxin[:, o : o + 500],
                        start=True, stop=True,
                    )
                    nc.scalar.copy(mxo[:, o : o + 500], psm[0:120, 0:500])
                a2 = a2pool.tile([100, 2 * H * P2], BF16)  # [keypos, (k,two,n)]
                for j in range(10):
                    eng = nc.sync if j % 2 == 0 else nc.gpsimd
                    eng.dma_start(a2[j * 10 : (j + 1) * 10, :],
                                  mxo[:, j * 400 : (j + 1) * 400])
                return a2

            def emit_back(b, a2):
                """AV + projection for one batch."""
                a2v = a2[:].rearrange("p (k two n) -> p k two n",
                                      k=H, two=2, n=P2)
                aoT = aopool.tile([128, KT * P2], BF16, tag="ao")
                for tt in range(H // 4):        # 3 psum tiles, 2 head-pairs each
                    pv = psMV.tile([128, 512], F32, tag="mv")
                    for jj2 in range(2):
                        jj = tt * 2 + jj2
                        for sub in range(2):
                            k = 2 * jj + sub
                            rows = pv[sub * 64 : sub * 64 + 64,
                                      jj2 * P2 : (jj2 + 1) * P2]
                            for c in range(2):
                                nc.tensor.matmul(
                                    rows,
                                    v_sb[0:100, (b * 2 + c) * 768 + k * 64 : (b * 2 + c) * 768 + (k + 1) * 64],
                                    a2v[0:100, k, c],
                                    start=(c == 0),
                                    stop=(c == 1),
                                    tile_position=(0, sub * 64),
                                )
                    nc.scalar.copy(aoT[:, tt * 2 * P2 : (tt + 1) * 2 * P2],
                                   pv[:, 0 : 2 * P2])

                for (t0, tsz) in [(0, 128), (128, 72)]:
                    osb = opool.tile([128, 768], BF16, tag="osb")
                    for (n0, nsz) in [(0, 512), (512, 256)]:
                        pp = psA.tile([128, 512], F32, tag="a")
                        nc.tensor.matmul(
                            pp[0:tsz, 0:nsz],
                            ones_row[0:1, 0:tsz],
                            bp[:, n0 : n0 + nsz],
                            start=True, stop=False,
                        )
                        for kt in range(KT):
                            nc.tensor.matmul(
                                pp[0:tsz, 0:nsz],
                                aoT[:, kt * P2 + t0 : kt * P2 + t0 + tsz],
                                wp[:, kt * 768 + n0 : kt * 768 + n0 + nsz],
                                start=False,
                                stop=(kt == KT - 1),
                            )
                        with nc.allow_low_precision(reason="output in bf16"):
                            nc.scalar.copy(osb[0:tsz, n0 : n0 + nsz], pp[0:tsz, 0:nsz])
                    nc.sync.dma_start(
                        out_d[b * P2 + t0 : b * P2 + t0 + tsz, :], osb[0:tsz, :]
                    )

            # 3-stage software-pipelined emission:
            #   f0 | f1 m0 | f2 b0 m1 | f3 b1 m2 | ... | b6 m7 | b7
            def emit_attention():
                exps = {}
                a2s = {}
                for b in range(BL):
                    expAll = exppool.tile([100, 2 * H * P2], BF16)
                    exps[b] = expAll
                    emit_front(b, expAll)
                    if b >= 2:
                        emit_back(b - 2, a2s.pop(b - 2))
                    if b >= 1:
                        a2s[b - 1] = emit_mix(exps.pop(b - 1))
                emit_back(BL - 2, a2s.pop(BL - 2))
                a2s[BL - 1] = emit_mix(exps.pop(BL - 1))
                emit_back(BL - 1, a2s.pop(BL - 1))

            exppool = smallpool = a2pool = mxpool = mopool = None
            aopool = dbpool = opool = None

            with tc.tile_pool(name="early", bufs=1) as epool:
                emit_stage12(epool)

            with (
                tc.tile_pool(name="exp", bufs=3) as exppool,
                tc.tile_pool(name="small", bufs=2) as smallpool,
                tc.tile_pool(name="a2", bufs=2) as a2pool,
                tc.tile_pool(name="mxin", bufs=2) as mxpool,
                tc.tile_pool(name="mxout", bufs=2) as mopool,
                tc.tile_pool(name="ao", bufs=2) as aopool,
                tc.tile_pool(name="denb2", bufs=2) as dbpool,
                tc.tile_pool(name="osb", bufs=2) as opool,
            ):
                emit_attention()

    nc.compile()
    return nc


def _tile6(a, width):
    """[768, M] -> [128, 6*M] (K-tile-major host layout)."""
    assert a.shape == (768, width)
    return np.ascontiguousarray(
        a.reshape(KT, 128, width).transpose(1, 0, 2).reshape(128, KT * width)
    )


def _to_bf16(a):
    return np.asarray(a, dtype=np.float32).astype(ml_dtypes.bfloat16)


def _posmaps():
    """token m -> padded position p, and p -> m (or -1 for dummies)."""
    pos_of_tok = np.empty(N, np.int64)
    for m in range(N):
        c = 0 if m < 100 else 1
        mm = m - c * 100
        g, ml = mm // 10, mm % 10
        pos_of_tok[m] = c * 100 + ml * 10 + g
    tok_of_pos = np.full(P2, -1, np.int64)
    tok_of_pos[pos_of_tok] = np.arange(N)
    return pos_of_tok, tok_of_pos


_POS_OF_TOK, _TOK_OF_POS = _posmaps()


def _preprocess(inputs):
    x = np.asarray(inputs["x"], np.float32)
    qkv_w = np.asarray(inputs["qkv_w"], np.float32)
    q_bias = np.asarray(inputs["q_bias"], np.float32)
    v_bias = np.asarray(inputs["v_bias"], np.float32)
    sq = np.asarray(inputs["ssf_scale_qkv"], np.float32)
    tq = np.asarray(inputs["ssf_shift_qkv"], np.float32)
    rbt = np.asarray(inputs["rel_bias_table"], np.float32)
    coeff = np.asarray(inputs["bases_coeff"], np.float32)
    proj_w = np.asarray(inputs["proj_w"], np.float32)
    proj_b = np.asarray(inputs["proj_b"], np.float32)
    sp = np.asarray(inputs["ssf_scale_proj"], np.float32)
    tp = np.asarray(inputs["ssf_shift_proj"], np.float32)
    rel_index = np.asarray(inputs["rel_index"], np.int64)

    qkv_bias = np.concatenate([q_bias, np.zeros_like(q_bias), v_bias])
    w_eff = (qkv_w * sq[:, None]).copy()
    b_eff = (qkv_bias * sq + tq).copy()
    w_eff[0:768] *= SCALE
    b_eff[0:768] *= SCALE

    wqk = _tile6(np.ascontiguousarray(w_eff[0:1536].T), 1536)
    wvt = _tile6(np.ascontiguousarray(w_eff[1536:].T), 768)
    wp_eff = proj_w * sp[:, None]
    bp_eff = proj_b * sp + tp
    wpt = _tile6(np.ascontiguousarray(wp_eff.T), 768)

    bqk_sb = np.ascontiguousarray(b_eff[0:1536].reshape(QKM, 128).T).astype(np.float32)

    # rel bias in permuted+padded coordinates:
    # relb[p, (h*2+c)*P2 + n] = table[rel_index[qtok(n), ktok(c,p)], h]
    # dummy keys get DUMMY_BIAS, dummy queries 0.
    gathered = rbt[rel_index]                      # [query-tok, key-tok, H]
    relb4 = np.zeros((100, H, 2, P2), np.float32)
    q_valid = _TOK_OF_POS >= 0                     # [P2]
    qtok = np.where(q_valid, _TOK_OF_POS, 0)
    for c in range(2):
        ktok_pos = _TOK_OF_POS[c * 100 : (c + 1) * 100]   # [100]
        k_valid = ktok_pos >= 0
        ktok = np.where(k_valid, ktok_pos, 0)
        blk = gathered[qtok[None, :], ktok[:, None], :]   # [100, P2, H]
        blk = blk.transpose(0, 2, 1)                      # [100, H, P2]
        blk = np.where(q_valid[None, None, :], blk, 0.0)
        blk = np.where(k_valid[:, None, None], blk, DUMMY_BIAS)
        relb4[:, :, c, :] = blk
    # upload exp(bias): the kernel multiplies exp(scores) by this instead
    # of adding the bias before the exp (dummy keys -> exp(-40) ~ 0).
    relb = np.exp(relb4.reshape(100, 2 * H * P2))

    # mix = coeff^T * 1.0 + I ; mixblk[wgi*12+h, wgi'*12+k] = d(wgi,wgi')mix[h,k]
    mix = coeff.T + np.eye(H, dtype=np.float32)
    mixblk = np.kron(np.eye(10, dtype=np.float32), mix)
    # softmax rows sum to 1, so the v-bias reaches the output as the constant
    # (sum_h mix[h,k]) * bv_k per head; fold it through Wp into the proj bias.
    s_k = mix.sum(axis=0)                               # [H]
    vb_fold = np.repeat(s_k, C // H) * b_eff[1536:]     # [768]
    bp_row = (bp_eff + vb_fold @ wp_eff.T).reshape(1, 768)

    common = {
        "wqk": _to_bf16(wqk),
        "wv": _to_bf16(wvt),
        "wp": _to_bf16(wpt),
        "relb": _to_bf16(relb),
        "mixblk": _to_bf16(mixblk),
        "bqk": bqk_sb,
        "bp": _to_bf16(bp_row),
    }
    in_maps = []
    for ci in range(NCORES):
        xs = x[ci * BL : (ci + 1) * BL]             # [BL, N, C]
        xp = np.zeros((BL, P2, C), np.float32)
        xp[:, _POS_OF_TOK, :] = xs
        xt = xp.reshape(BL * P2, C).T               # [C, T2]
        m = dict(common)
        m["xT"] = _to_bf16(_tile6(np.ascontiguousarray(xt), T2))
        in_maps.append(m)
    return in_maps


def _get_compiled():
    if "nc" not in _COMPILED:
        _COMPILED["nc"] = _build_graph()
    return _COMPILED["nc"]


LAST_EXEC_NS = None
LAST_RESULTS = None


def _ensure_ntff_hook():
    """The agent image's antenv package lacks axon_hooks; synthesize it so
    run_bass_kernel_spmd(trace=True) can capture NTFF profiles."""
    import types

    if "antenv.axon_hooks" in sys.modules:
        return
    try:
        sys.path.insert(0, "/root/.axon_site")
        from trn_agent_boot.trn_boot import _ntff_profile_via_ctypes

        hook = _ntff_profile_via_ctypes("/opt/axon/libaxon_pjrt.so")
    except Exception:
        hook = None
    mod = types.ModuleType("antenv.axon_hooks")
    _state = {"hook": hook}
    mod.get_axon_ntff_profile_hook = lambda: _state["hook"]
    mod.set_axon_ntff_profile_hook = lambda h: _state.__setitem__("hook", h)
    sys.modules["antenv.axon_hooks"] = mod


def kernel(**inputs) -> np.ndarray:
    global LAST_EXEC_NS, LAST_RESULTS
    nc = _get_compiled()
    in_maps = _preprocess(inputs)
    from concourse.bass_utils import run_bass_kernel_spmd

    trace = os.environ.get("BASS_KERNEL_PROFILE", "0") == "1"
    if trace:
        _ensure_ntff_hook()
    res = run_bass_kernel_spmd(nc, in_maps, core_ids=list(range(NCORES)), trace=trace)
    LAST_EXEC_NS = res.exec_time_ns
    LAST_RESULTS = res
    outs = []
    for i in range(NCORES):
        o = np.asarray(res.results[i]["out"], np.float32).reshape(BL, P2, C)
        outs.append(o[:, _POS_OF_TOK, :])           # drop dummies, un-permute
    return np.concatenate(outs, axis=0).astype(np.float32)


# revision 24
# speedup vs baseline: 1.2351x; 1.0064x over previous
"""Trainium2 Bass kernel for nn_Attention_39608188404100.

Windowed-attention block (ViT-style, N=197 tokens) with SSF affines, relative
position bias, DCF head mixing, and output projection.

Strategy: pure data-parallel over batch across 8 NeuronCores (B=64 -> 8/core).
All weights replicated; no collectives. Compute in bf16 on the TensorEngine
(fp32 PSUM accumulation). fp8 was tried and rejected: each e4m3 quantization
point adds ~3.6% RMS error, far over the 2% tolerance.

Per core (BL=8 batches): each batch's 197 tokens are padded to 200 positions
and PERMUTED on host: position p = c*100 + ml*10 + g holds token
m = c*100 + g*10 + ml (c = chunk, 2x100). The 3 dummy positions per batch get
zero x-columns and a -40 relative-bias on their key rows; dummy query columns
are dropped on host after download.

Pipeline: dense QKV+V phase first (TensorEngine at ~95%), then a per-batch
attention loop emitted in an explicit 3-stage software-pipelined order
(front(b+2), back(b), mix(b+1)) so the scheduler overlaps three batches'
chains: scores/exp/softmax-epilogue (front), keypos<->(wgi,h) shuffle +
block-diagonal [120x120] DCF mix (mix), AV + projection (back). Shuffles are
10 direct SBUF->SBUF DMAs per direction per batch (plain-slice APs, 800B
descriptors).

Softmax denominator: ones-column matmul into PSUM, in-place fp32
reciprocal_approx_fast on the PSUM row, bf16 row, DRAM-bounce broadcast DMA
to 100 partitions, two in-place normalize multiplies.
"""
import os
import sys

sys.path.insert(0, "/opt/trn_rl_repo")

import numpy as np
import ml_dtypes

import concourse.bass as bass
import concourse.tile as tile
from concourse import bacc, mybir

BF16 = mybir.dt.bfloat16
F32 = mybir.dt.float32
AF = mybir.ActivationFunctionType
ALU = mybir.AluOpType

B, N, C, H = 64, 197, 768, 12
NCORES = 8
BL = B // NCORES          # 8 batches per core
P2 = 200                  # padded positions per batch
T2 = BL * P2              # 1600 positions per core
PW = 2 * P2               # pair width (400 token columns)
SCALE = (C // H) ** -0.5
KT = 6                    # contraction tiles of 128 over C=768
QKM = 12                  # 128-wide M tiles over 1536 q/k channels
DUMMY_BIAS = -40.0
DEN_CH = 480              # denominator column chunk (5 x 480 = H*P2)

_COMPILED = {}


def _build_graph():
    nc = bacc.Bacc(
        "TRN2", target_bir_lowering=False, debug=False,
        detect_race_conditions=False,
    )

    xT_d = nc.dram_tensor("xT", [128, KT * T2], BF16, kind="ExternalInput")
    wqk_d = nc.dram_tensor("wqk", [128, KT * 1536], BF16, kind="ExternalInput")
    wv_d = nc.dram_tensor("wv", [128, KT * 768], BF16, kind="ExternalInput")
    wp_d = nc.dram_tensor("wp", [128, KT * 768], BF16, kind="ExternalInput")
    relb_d = nc.dram_tensor("relb", [100, 2 * H * P2], BF16, kind="ExternalInput")
    mix_d = nc.dram_tensor("mixblk", [120, 120], BF16, kind="ExternalInput")
    bqk_d = nc.dram_tensor("bqk", [128, QKM], F32, kind="ExternalInput")
    bp_d = nc.dram_tensor("bp", [1, 768], BF16, kind="ExternalInput")
    out_d = nc.dram_tensor("out", [T2, 768], BF16, kind="ExternalOutput")

    with tile.TileContext(nc) as tc:
        with (
            tc.tile_pool(name="const", bufs=1) as cpool,
            tc.tile_pool(name="dram", bufs=2, space=bass.MemorySpace.DRAM) as drpool,
            tc.tile_pool(name="psA", bufs=2, space=bass.MemorySpace.PSUM) as psA,
            tc.tile_pool(name="psS", bufs=2, space=bass.MemorySpace.PSUM) as psS,
            tc.tile_pool(name="psD", bufs=1, space=bass.MemorySpace.PSUM) as psD,
            tc.tile_pool(name="psMV", bufs=3, space=bass.MemorySpace.PSUM) as psMV,
        ):
            # ---- persistent constants / weights ----
            wp = cpool.tile([128, KT * 768], BF16)
            relb = cpool.tile([100, 2 * H * P2], BF16)
            mixblk = cpool.tile([120, 120], BF16)
            bqk = cpool.tile([128, QKM], F32)
            bp = cpool.tile([1, 768], BF16)
            ones_col = cpool.tile([128, 1], BF16)   # lhsT for denominator
            ones_row = cpool.tile([1, 128], BF16)   # lhsT for rank-1 bias
            nc.sync.dma_start(wp[:], wp_d[:])
            nc.sync.dma_start(relb[:], relb_d[:])
            nc.sync.dma_start(mixblk[:], mix_d[:])
            nc.sync.dma_start(bqk[:], bqk_d[:])
            nc.sync.dma_start(bp[:], bp_d[:])
            nc.vector.memset(ones_col[:], 1.0)
            nc.vector.memset(ones_row[:], 1.0)

            qk_sb = cpool.tile([128, QKM * T2], BF16)      # qkT: [ch-tile, pos]
            v_sb = cpool.tile([100, 2 * BL * 768], BF16)   # v: [pos, (b,c)*768+ch]

            def emit_stage12(epool):
                xT = epool.tile([128, KT * T2], BF16)
                wqk = epool.tile([128, KT * 1536], BF16)
                wv = epool.tile([128, KT * 768], BF16)
                for kt in range(KT):
                    nc.sync.dma_start(
                        wqk[:, kt * 1536 : (kt + 1) * 1536],
                        wqk_d[:, kt * 1536 : (kt + 1) * 1536],
                    )
                for half in range(2):
                    h0 = half * (T2 // 2)
                    for kt in range(KT):
                        nc.sync.dma_start(
                            xT[:, kt * T2 + h0 : kt * T2 + h0 + T2 // 2],
                            xT_d[:, kt * T2 + h0 : kt * T2 + h0 + T2 // 2],
                        )
                nc.sync.dma_start(wv[:], wv_d[:])

                for mt in range(QKM):
                    for n0 in range(0, T2, 400):
                        ps = psA.tile([128, 512], F32, tag="a")
                        for kt in range(KT):
                            nc.tensor.matmul(
                                ps[:, 0:400],
                                wqk[:, kt * 1536 + mt * 128 : kt * 1536 + (mt + 1) * 128],
                                xT[:, kt * T2 + n0 : kt * T2 + n0 + 400],
                                start=(kt == 0),
                                stop=(kt == KT - 1),
                            )
                        nc.scalar.activation(
                            qk_sb[:, mt * T2 + n0 : mt * T2 + n0 + 400],
                            ps[:, 0:400],
                            AF.Identity,
                            bias=bqk[:, mt : mt + 1],
                            scale=1.0,
                        )

                # v carries NO bias: since softmax rows sum to exactly 1,
                # the v-bias term through the mixed attention is the constant
                # (sum_h mix[h,k]) * bv_k per head -- folded into bp on host.
                for b in range(BL):
                    for c in range(2):
                        base = b * P2 + c * 100
                        for (n0, nsz) in [(0, 512), (512, 256)]:
                            ps = psA.tile([128, 512], F32, tag="a")
                            for kt in range(KT):
                                nc.tensor.matmul(
                                    ps[0:100, 0:nsz],
                                    xT[:, kt * T2 + base : kt * T2 + base + 100],
                                    wv[:, kt * 768 + n0 : kt * 768 + n0 + nsz],
                                    start=(kt == 0),
                                    stop=(kt == KT - 1),
                                )
                            nc.scalar.copy(
                                v_sb[0:100, (b * 2 + c) * 768 + n0 : (b * 2 + c) * 768 + n0 + nsz],
                                ps[0:100, 0:nsz],
                            )

            def emit_front(b, expAll):
                """Scores, exp, softmax epilogue, in-place normalize."""
                eP = expAll[:].rearrange("p (h two n) -> p h two n",
                                         h=H, two=2, n=P2)
                for h in range(H):
                    prow = (h % 2) * 64
                    qoff = (h // 2) * T2 + b * P2
                    koff = (6 + h // 2) * T2 + b * P2

                    ps1 = psS.tile([128, 512], F32, tag="s")
                    nc.tensor.matmul(
                        ps1[0:100, 0:P2],
                        qk_sb[prow : prow + 64, koff : koff + 100],
                        qk_sb[prow : prow + 64, qoff : qoff + P2],
                        start=True, stop=True,
                    )
                    nc.tensor.matmul(
                        ps1[0:100, P2 : 2 * P2],
                        qk_sb[prow : prow + 64, koff + 100 : koff + 200],
                        qk_sb[prow : prow + 64, qoff : qoff + P2],
                        start=True, stop=True,
                    )
                    nc.scalar.activation(eP[0:100, h], ps1[0:100, 0 : 2 * P2],
                                         AF.Exp)

                # softmax epilogue: exp(rel-bias) multiply + chunk-sum in two
                # head-halves, ones-matmul denominator per 480-col chunk,
                # in-place fp32 fast reciprocal in PSUM, bf16 row, DRAM-bounce
                # broadcast, two in-place normalize multiplies.
                ev = eP[0:100]                              # [100, h, two, n]
                rv = relb[0:100, :].rearrange("p (h c n) -> p h c n", h=H, c=2, n=P2)
                denb = smallpool.tile([100, H * P2], BF16, tag="denb")
                dv = denb[:].rearrange("p (h n) -> p h n", h=H)
                for h0 in (0, 6):
                    nc.vector.tensor_tensor(ev[:, h0 : h0 + 6], ev[:, h0 : h0 + 6],
                                            rv[:, h0 : h0 + 6], ALU.mult)
                    nc.vector.tensor_tensor(dv[:, h0 : h0 + 6],
                                            ev[:, h0 : h0 + 6, 0, :],
                                            ev[:, h0 : h0 + 6, 1, :], ALU.add)
                denr = dbpool.tile([1, H * P2], BF16, tag="denr")
                for o in range(0, H * P2, DEN_CH):
                    psd = psD.tile([128, 512], F32, tag="d")
                    nc.tensor.matmul(psd[0:1, 0:DEN_CH], ones_col[0:100, 0:1],
                                     denb[0:100, o : o + DEN_CH],
                                     start=True, stop=True)
                    nc.vector.reciprocal_approx_fast(psd[0:1, 0:DEN_CH],
                                                     psd[0:1, 0:DEN_CH])
                    with nc.allow_low_precision(reason="softmax recip in bf16"):
                        nc.vector.tensor_copy(denr[0:1, o : o + DEN_CH],
                                              psd[0:1, 0:DEN_CH])
                ddr = drpool.tile([1, H * P2], BF16, tag="ddr")
                nc.gpsimd.dma_start(ddr[:], denr[:])
                denbc = dbpool.tile([100, H * P2], BF16, tag="denbc")
                nc.gpsimd.dma_start(denbc[:], ddr[:].to_broadcast([100, H * P2]))
                dvb = denbc[:].rearrange("p (h n) -> p h n", h=H)
                nc.vector.tensor_tensor(ev[:, :, 0, :], ev[:, :, 0, :], dvb, ALU.mult)
                nc.vector.tensor_tensor(ev[:, :, 1, :], ev[:, :, 1, :], dvb, ALU.mult)

            def emit_mix(expAll):
                """keypos->(wgi,h) shuffle, block-diag mix, shuffle back.
                mxin[wgi*12+h, j*400+(two n)] = expAll[j*10+wgi, h*400+(two n)]"""
                mxin = mxpool.tile([120, 10 * PW], BF16, tag="mxin")
                for j in range(10):
                    eng = nc.sync if j % 2 == 0 else nc.gpsimd
                    eng.dma_start(mxin[:, j * 400 : (j + 1) * 400],
                                  expAll[j * 10 : (j + 1) * 10, :])
                mxo = mopool.tile([120, 10 * PW], BF16, tag="mxout")
                for o in range(0, 4000, 500):
                    psm = psMV.tile([128, 512], F32, tag="mv")
                    nc.tensor.matmul(
                        psm[0:120, 0:500], mixblk[:],
                        mxin[:, o : o + 500],
                        start=True, stop=True,
                    )
                    nc.scalar.copy(mxo[:, o : o + 500], psm[0:120, 0:500])
                a2 = a2pool.tile([100, 2 * H * P2], BF16)  # [keypos, (k,two,n)]
                for j in range(10):
                    eng = nc.sync if j % 2 == 0 else nc.gpsimd
                    eng.dma_start(a2[j * 10 : (j + 1) * 10, :],
                                  mxo[:, j * 400 : (j + 1) * 400])
                return a2

            def emit_back(b, a2):
                """AV + projection for one batch."""
                a2v = a2[:].rearrange("p (k two n) -> p k two n",
                                      k=H, two=2, n=P2)
                aoT = aopool.tile([128, KT * P2], BF16, tag="ao")
                for tt in range(H // 4):        # 3 psum tiles, 2 head-pairs each
                    pv = psMV.tile([128, 512], F32, tag="mv")
                    for jj2 in range(2):
                        jj = tt * 2 + jj2
                        for sub in range(2):
                            k = 2 * jj + sub
                            rows = pv[sub * 64 : sub * 64 + 64,
                                      jj2 * P2 : (jj2 + 1) * P2]
                            for c in range(2):
                                nc.tensor.matmul(
                                    rows,
                                    v_sb[0:100, (b * 2 + c) * 768 + k * 64 : (b * 2 + c) * 768 + (k + 1) * 64],
                                    a2v[0:100, k, c],
                                    start=(c == 0),
                                    stop=(c == 1),
                                    tile_position=(0, sub * 64),
                                )
                    nc.scalar.copy(aoT[:, tt * 2 * P2 : (tt + 1) * 2 * P2],
                                   pv[:, 0 : 2 * P2])

                for (t0, tsz) in [(0, 128), (128, 72)]:
                    osb = opool.tile([128, 768], BF16, tag="osb")
                    for (n0, nsz) in [(0, 512), (512, 256)]:
                        pp = psA.tile([128, 512], F32, tag="a")
                        nc.tensor.matmul(
                            pp[0:tsz, 0:nsz],
                            ones_row[0:1, 0:tsz],
                            bp[:, n0 : n0 + nsz],
                            start=True, stop=False,
                        )
                        for kt in range(KT):
                            nc.tensor.matmul(
                                pp[0:tsz, 0:nsz],
                                aoT[:, kt * P2 + t0 : kt * P2 + t0 + tsz],
                                wp[:, kt * 768 + n0 : kt * 768 + n0 + nsz],
                                start=False,
                                stop=(kt == KT - 1),
                            )
                        with nc.allow_low_precision(reason="output in bf16"):
                            nc.scalar.copy(osb[0:tsz, n0 : n0 + nsz], pp[0:tsz, 0:nsz])
                    nc.sync.dma_start(
                        out_d[b * P2 + t0 : b * P2 + t0 + tsz, :], osb[0:tsz, :]
                    )

            # 3-stage software-pipelined emission:
            #   f0 | f1 m0 | f2 b0 m1 | f3 b1 m2 | ... | b6 m7 | b7
            def emit_attention():
                exps = {}
                a2s = {}
                for b in range(BL):
                    expAll = exppool.tile([100, 2 * H * P2], BF16)
                    exps[b] = expAll
                    emit_front(b, expAll)
                    if b >= 2:
                        emit_back(b - 2, a2s.pop(b - 2))
                    if b >= 1:
                        a2s[b - 1] = emit_mix(exps.pop(b - 1))
                emit_back(BL - 2, a2s.pop(BL - 2))
                a2s[BL - 1] = emit_mix(exps.pop(BL - 1))
                emit_back(BL - 1, a2s.pop(BL - 1))

            exppool = smallpool = a2pool = mxpool = mopool = None
            aopool = dbpool = opool = None

            with tc.tile_pool(name="early", bufs=1) as epool:
                emit_stage12(epool)

            with (
                tc.tile_pool(name="exp", bufs=4) as exppool,
                tc.tile_pool(name="small", bufs=2) as smallpool,
                tc.tile_pool(name="a2", bufs=2) as a2pool,
                tc.tile_pool(name="mxin", bufs=2) as mxpool,
                tc.tile_pool(name="mxout", bufs=2) as mopool,
                tc.tile_pool(name="ao", bufs=2) as aopool,
                tc.tile_pool(name="denb2", bufs=2) as dbpool,
                tc.tile_pool(name="osb", bufs=2) as opool,
            ):
                emit_attention()

    nc.compile()
    return nc


def _tile6(a, width):
    """[768, M] -> [128, 6*M] (K-tile-major host layout)."""
    assert a.shape == (768, width)
    return np.ascontiguousarray(
        a.reshape(KT, 128, width).transpose(1, 0, 2).reshape(128, KT * width)
    )


def _to_bf16(a):
    return np.asarray(a, dtype=np.float32).astype(ml_dtypes.bfloat16)


def _posmaps():
    """token m -> padded position p, and p -> m (or -1 for dummies)."""
    pos_of_tok = np.empty(N, np.int64)
    for m in range(N):
        c = 0 if m < 100 else 1
        mm = m - c * 100
        g, ml = mm // 10, mm % 10
        pos_of_tok[m] = c * 100 + ml * 10 + g
    tok_of_pos = np.full(P2, -1, np.int64)
    tok_of_pos[pos_of_tok] = np.arange(N)
    return pos_of_tok, tok_of_pos


_POS_OF_TOK, _TOK_OF_POS = _posmaps()


def _preprocess(inputs):
    x = np.asarray(inputs["x"], np.float32)
    qkv_w = np.asarray(inputs["qkv_w"], np.float32)
    q_bias = np.asarray(inputs["q_bias"], np.float32)
    v_bias = np.asarray(inputs["v_bias"], np.float32)
    sq = np.asarray(inputs["ssf_scale_qkv"], np.float32)
    tq = np.asarray(inputs["ssf_shift_qkv"], np.float32)
    rbt = np.asarray(inputs["rel_bias_table"], np.float32)
    coeff = np.asarray(inputs["bases_coeff"], np.float32)
    proj_w = np.asarray(inputs["proj_w"], np.float32)
    proj_b = np.asarray(inputs["proj_b"], np.float32)
    sp = np.asarray(inputs["ssf_scale_proj"], np.float32)
    tp = np.asarray(inputs["ssf_shift_proj"], np.float32)
    rel_index = np.asarray(inputs["rel_index"], np.int64)

    qkv_bias = np.concatenate([q_bias, np.zeros_like(q_bias), v_bias])
    w_eff = (qkv_w * sq[:, None]).copy()
    b_eff = (qkv_bias * sq + tq).copy()
    w_eff[0:768] *= SCALE
    b_eff[0:768] *= SCALE

    wqk = _tile6(np.ascontiguousarray(w_eff[0:1536].T), 1536)
    wvt = _tile6(np.ascontiguousarray(w_eff[1536:].T), 768)
    wp_eff = proj_w * sp[:, None]
    bp_eff = proj_b * sp + tp
    wpt = _tile6(np.ascontiguousarray(wp_eff.T), 768)

    bqk_sb = np.ascontiguousarray(b_eff[0:1536].reshape(QKM, 128).T).astype(np.float32)

    # rel bias in permuted+padded coordinates:
    # relb[p, (h*2+c)*P2 + n] = table[rel_index[qtok(n), ktok(c,p)], h]
    # dummy keys get DUMMY_BIAS, dummy queries 0.
    gathered = rbt[rel_index]                      # [query-tok, key-tok, H]
    relb4 = np.zeros((100, H, 2, P2), np.float32)
    q_valid = _TOK_OF_POS >= 0                     # [P2]
    qtok = np.where(q_valid, _TOK_OF_POS, 0)
    for c in range(2):
        ktok_pos = _TOK_OF_POS[c * 100 : (c + 1) * 100]   # [100]
        k_valid = ktok_pos >= 0
        ktok = np.where(k_valid, ktok_pos, 0)
        blk = gathered[qtok[None, :], ktok[:, None], :]   # [100, P2, H]
        blk = blk.transpose(0, 2, 1)                      # [100, H, P2]
        blk = np.where(q_valid[None, None, :], blk, 0.0)
        blk = np.where(k_valid[:, None, None], blk, DUMMY_BIAS)
        relb4[:, :, c, :] = blk
    # upload exp(bias): the kernel multiplies exp(scores) by this instead
    # of adding the bias before the exp (dummy keys -> exp(-40) ~ 0).
    relb = np.exp(relb4.reshape(100, 2 * H * P2))

    # mix = coeff^T * 1.0 + I ; mixblk[wgi*12+h, wgi'*12+k] = d(wgi,wgi')mix[h,k]
    mix = coeff.T + np.eye(H, dtype=np.float32)
    mixblk = np.kron(np.eye(10, dtype=np.float32), mix)
    # softmax rows sum to 1, so the v-bias reaches the output as the constant
    # (sum_h mix[h,k]) * bv_k per head; fold it through Wp into the proj bias.
    s_k = mix.sum(axis=0)                               # [H]
    vb_fold = np.repeat(s_k, C // H) * b_eff[1536:]     # [768]
    bp_row = (bp_eff + vb_fold @ wp_eff.T).reshape(1, 768)

    common = {
        "wqk": _to_bf16(wqk),
        "wv": _to_bf16(wvt),
        "wp": _to_bf16(wpt),
        "relb": _to_bf16(relb),
        "mixblk": _to_bf16(mixblk),
        "bqk": bqk_sb,
        "bp": _to_bf16(bp_row),
    }
    in_maps = []
    for ci in range(NCORES):
        xs = x[ci * BL : (ci + 1) * BL]             # [BL, N, C]
        xp = np.zeros((BL, P2, C), np.float32)
        xp[:, _POS_OF_TOK, :] = xs
        xt = xp.reshape(BL * P2, C).T               # [C, T2]
        m = dict(common)
        m["xT"] = _to_bf16(_tile6(np.ascontiguousarray(xt), T2))
        in_maps.append(m)
    return in_maps


def _get_compiled():
    if "nc" not in _COMPILED:
        _COMPILED["nc"] = _build_graph()
    return _COMPILED["nc"]


LAST_EXEC_NS = None
LAST_RESULTS = None


def _ensure_ntff_hook():
    """The agent image's antenv package lacks axon_hooks; synthesize it so
    run_bass_kernel_spmd(trace=True) can capture NTFF profiles."""
    import types

    if "antenv.axon_hooks" in sys.modules:
        return
    try:
        sys.path.insert(0, "/root/.axon_site")
        from trn_agent_boot.trn_boot import _ntff_profile_via_ctypes

        hook = _ntff_profile_via_ctypes("/opt/axon/libaxon_pjrt.so")
    except Exception:
        hook = None
    mod = types.ModuleType("antenv.axon_hooks")
    _state = {"hook": hook}
    mod.get_axon_ntff_profile_hook = lambda: _state["hook"]
    mod.set_axon_ntff_profile_hook = lambda h: _state.__setitem__("hook", h)
    sys.modules["antenv.axon_hooks"] = mod


def kernel(**inputs) -> np.ndarray:
    global LAST_EXEC_NS, LAST_RESULTS
    nc = _get_compiled()
    in_maps = _preprocess(inputs)
    from concourse.bass_utils import run_bass_kernel_spmd

    trace = os.environ.get("BASS_KERNEL_PROFILE", "0") == "1"
    if trace:
        _ensure_ntff_hook()
    res = run_bass_kernel_spmd(nc, in_maps, core_ids=list(range(NCORES)), trace=trace)
    LAST_EXEC_NS = res.exec_time_ns
    LAST_RESULTS = res
    outs = []
    for i in range(NCORES):
        o = np.asarray(res.results[i]["out"], np.float32).reshape(BL, P2, C)
        outs.append(o[:, _POS_OF_TOK, :])           # drop dummies, un-permute
    return np.concatenate(outs, axis=0).astype(np.float32)
